# revision 66
# baseline (speedup 1.0000x reference)
"""GAT spatial kernel for trn2 (nn_GATSpatial_36112085025002) — v2.

Strategy
--------
Data-parallel over B=8 across the 8 NeuronCores; each core runs the full
2-layer GAT for one batch element.

v2 design (vs v1 baseline at ~455us):
  - ACT engine does ONLY the exp over the N^2 scores (the hard floor);
    all copies move to DVE/Pool/DMA.
  - Scores in transposed layout sT[keys, q] via K=65 augmented contraction:
    stationary aug_k rows 0-63 = h^T fp16, row 64 = ones (produced for free
    by an augmented projection: x^T gets a ones row, Wh^T gets a selector
    column); moving aug_q rows 0-63 = h^T, row 64 = -||h_q||^2 (softmax
    shift; exact by shift-invariance, so fp16 precision is fine).
  - -r^2 per chunk via one fused tensor_tensor_reduce off the PV-stationary
    tile, then PE-transpose + one strided DMA into aug_q row 64.
  - H_aug (PV stationary, [128, 65] per chunk incl ones col for free
    denominators) built with DMA-xbar transposes — zero PSUM traffic, so
    the whole L1 aug prep runs in a prologue overlapped with the mask DMA
    and the attention passes run back-to-back with no PE gaps (keeps the
    PE p-state ramped).
  - exp: ACT [128,1024] PSUM->bf16; mask as bf16 multiply on DVE (2x mode).
  - PSUM: scores [128,1024]x2 + out accum [65,1024]x2 = exactly 8 banks.
  - L1 normalization: denominators DMA-roundtrip (DRAM gather -> 128-lane
    reciprocal -> broadcast), numerators scaled straight out of PSUM on DVE.
  - L2 epilogue: numerators -> bf16 -> DMA-xbar transpose to [q, d], per-
    block reciprocal from a gathered [128, 8] column layout, leaky+LN on
    DVE with batched sqrt on ACT.
"""
import sys

sys.path.insert(0, '/opt/trn_rl_repo')

import numpy as np
import ml_dtypes

import concourse.bass as bass
import concourse.tile as tile
import concourse.mybir as mybir
from concourse.masks import make_identity

F32 = mybir.dt.float32
F32R = mybir.dt.float32r
F16 = mybir.dt.float16
BF16 = mybir.dt.bfloat16
AF = mybir.ActivationFunctionType
ALU = mybir.AluOpType
AX = mybir.AxisListType

N_CORES = 8
LN_EPS = 1e-5

# ---------------------------------------------------------------------------
# walrus workaround: this compiler build rejects >1 sync-wait per instruction.
# Split extra waits into standalone EventSemaphore instructions.
# ---------------------------------------------------------------------------
_orig_commit = tile.TileContext._commit_and_lower


def _patched_commit(self, inst, *args, **kwargs):
    si = getattr(inst, "sync_info", None)
    waits = list(si.on_wait) if si is not None and si.on_wait else []
    if len(waits) > 1:
        for w in waits[:-1]:
            ev = mybir.InstEventSemaphore(
                name=self.nc.get_next_instruction_name(),
                engine=inst.engine,
                ins=[],
                outs=[],
                sync_info=mybir.SyncInfo(on_wait=[w], on_update=[]),
            )
            _orig_commit(self, ev, *args, **kwargs)
        si.on_wait = [waits[-1]]
        inst.sync_info = si
    return _orig_commit(self, inst, *args, **kwargs)


def _patched_drain_and_barrier(self, tick_clock, wait_clock):
    from concourse.tile import ScopedClock

    nc = self.nc
    dummy = mybir.InstDrain(
        name="tail-drain-waits", ins=[], outs=[], bass_is_fusable=False
    )
    dummy.engine = nc.sync.engine
    wait_clock.add_sem_waits(dummy, ScopedClock({None: tick_clock.global_clock}))
    waits = list(dummy.sync_info.on_wait) if dummy.sync_info else []
    for w in waits:
        ev = mybir.InstEventSemaphore(
            name=nc.get_next_instruction_name(),
            engine=nc.sync.engine,
            ins=[],
            outs=[],
            sync_info=mybir.SyncInfo(on_wait=[w], on_update=[]),
        )
        nc.sync.add_instruction(ev)
    nc.sync.drain()

    nc.all_engine_barrier()
    assert self.sems is not None
    popped = nc._tile_sem_poison_stack.pop()
    assert popped is self._sem_poison
    nc.clear_and_free_semaphores(list(self.sems.allocated().values()))
    nc.all_engine_barrier()


if getattr(tile.TileContext, "_wait_split_patched", False) is False:
    tile.TileContext._commit_and_lower = _patched_commit
    tile.TileContext._drain_and_barrier = _patched_drain_and_barrier
    tile.TileContext._wait_split_patched = True


# ---------------------------------------------------------------------------
# Kernel builder
# ---------------------------------------------------------------------------
def build_gat(N=2048, C=64, H=4, D=64,
              use_bh=False, use_bo=False, use_gamma=False, use_beta=False):
    assert N % 128 == 0
    NT = N // 128                     # key chunks
    QB = 1024                         # q block
    NQB = N // QB
    HD = H * D
    CHW = 65                          # H_aug per-chunk column stride (64 + ones)

    nc = bass.Bass(trn_type="TRN2")
    xt_d = nc.dram_tensor("xt", [C + 1, N], F32R, kind="ExternalInput")
    maskt_d = nc.dram_tensor("maskt", [N, N], BF16, kind="ExternalInput")
    wht_d = nc.dram_tensor("wht", [C + 1, H * 66], F32R, kind="ExternalInput")
    negr2_d = nc.dram_tensor("negr2", [H, N], F16, kind="ExternalInput")
    wot_d = nc.dram_tensor("wot", [128, (HD // 128) * D], F32R, kind="ExternalInput")
    bh_d = nc.dram_tensor("bh", [128, HD // 128], F32, kind="ExternalInput") if use_bh else None
    bo_d = nc.dram_tensor("bo", [D], F32, kind="ExternalInput") if use_bo else None
    gamma_d = nc.dram_tensor("gamma", [D], F32, kind="ExternalInput") if use_gamma else None
    beta_d = nc.dram_tensor("beta", [D], F32, kind="ExternalInput") if use_beta else None
    out_d = nc.dram_tensor("out", [N, D], F32, kind="ExternalOutput")

    with tile.TileContext(nc) as tc:
        import contextlib
        ctx = contextlib.ExitStack()
        with ctx:
            const = ctx.enter_context(tc.tile_pool(name="const", bufs=1))
            aug = ctx.enter_context(tc.tile_pool(name="aug", bufs=3))
            work = ctx.enter_context(tc.tile_pool(name="work", bufs=2))
            small = ctx.enter_context(tc.tile_pool(name="small", bufs=4))
            ppool = ctx.enter_context(tc.tile_pool(name="ppool", bufs=4))
            pss = ctx.enter_context(tc.tile_pool(name="pss", bufs=2, space="PSUM"))
            drb = ctx.enter_context(tc.tile_pool(name="drb", bufs=4, space="DRAM"))
            pso = ctx.enter_context(tc.tile_pool(name="pso", bufs=2, space="PSUM"))

            # ---- constants ----------------------------------------------------
            idf32 = const.tile([128, 128], F32, name="idf32")
            make_identity(nc, idf32)
            idf16 = const.tile([128, 128], F16, name="idf16")
            nc.vector.tensor_copy(idf16, idf32)
            eps_col = const.tile([128, 1], F32, name="eps_col")
            nc.vector.memset(eps_col, LN_EPS * D)
            onesrow = const.tile([1, N], F16, name="onesrow")
            nc.gpsimd.memset(onesrow, 1.0)

            xT = const.tile([C + 1, N], F32R, name="xT")
            nc.sync.dma_start(xT, xt_d[:, :])
            whT_sb = const.tile([C + 1, H * 66], F32R, name="whT_sb")
            nc.sync.dma_start(whT_sb, wht_d[:, :])
            woT_sb = const.tile([128, 2 * D], F32R, name="woT_sb")
            nc.sync.dma_start(woT_sb, wot_d[:, :])

            # mask resident in SBUF: [128, NT*N] bf16, chunk mc at cols
            # [mc*N, (mc+1)*N); one DMA per chunk, split across both hwdge
            # queues (SP + ACT) in consumption order. Chunks 2+ are emitted
            # after the head-0 aug build so its r64 DMA isn't queued behind
            # them on SP (see below).
            mask_sb = const.tile([128, NT * N], BF16, name="mask_sb")

            def mask_dma(mc):
                eng = nc.sync if mc % 2 == 0 else nc.scalar
                eng.dma_start(mask_sb[:, mc * N:(mc + 1) * N],
                              maskt_d[mc * 128:(mc + 1) * 128, :])
            mask_dma(0)
            mask_dma(1)

            bh_cols = None
            if use_bh:
                bh_cols = const.tile([128, 2], F32, name="bh_cols")
                nc.sync.dma_start(bh_cols, bh_d[:, :])
            bo_row = gamma_row = beta_row = None
            if use_bo:
                bo_row = const.tile([128, D], F32, name="bo_row")
                nc.sync.dma_start(bo_row, bo_d.to_broadcast([128, D]))
            if use_gamma:
                gamma_row = const.tile([128, D], F32, name="gamma_row")
                nc.sync.dma_start(gamma_row, gamma_d.to_broadcast([128, D]))
            if use_beta:
                beta_row = const.tile([128, D], F32, name="beta_row")
                nc.sync.dma_start(beta_row, beta_d.to_broadcast([128, D]))

            # ---- aug builder --------------------------------------------------
            HNT = NT // 2             # chunks per half

            def make_aug(tag, full65, r64_eng=None, host_r2_row=None,
                         dve_k=False):
                """Returns ((aug_q, aug_k, H_aug), half_units). half_units(
                half, proj_thunk, hmk_thunk) -> list of small thunks building
                columns [half*QB,(half+1)*QB): d-major proj -> aug copies;
                key-major H chunks straight from mini-matmuls (no transposes);
                fused -r^2 per chunk; strided DMA into aug_q row 64. Thunk-
                granular so the work interleaves into a running pass."""
                aug_q = aug.tile([65, N], F16, name=f"aq_{tag}", tag="aug_q")
                aug_k = aug.tile([65, N], F16, name=f"ak_{tag}", tag="aug_k")
                H_aug = aug.tile([128, CHW * NT], F16, name=f"Ha_{tag}", tag="H_aug")
                rows = 65 if full65 else 64
                hw = 66 if full65 else 64   # k-major matmul output width
                # (66: f32r moving operands need an even free size; col 64 is
                # the ones column, col 65 zero padding)

                def half_units(half, proj_thunk, hmk_thunk):
                    j0 = half * QB
                    st = {}

                    def u_proj():
                        if half == 0:
                            if host_r2_row is not None:
                                # host-precomputed -r^2 straight into row 64
                                nc.sync.dma_start(aug_q[64:65, :], host_r2_row)
                            nc.sync.dma_start(aug_k[64:65, :], onesrow)
                        ps = proj_thunk()
                        # GPSIMD can't read PSUM: aug_q from PSUM on DVE,
                        # aug_k mirrored from aug_q on Pool (SBUF->SBUF).
                        nc.vector.tensor_copy(aug_q[0:64, j0:j0 + QB],
                                              ps[0:64, :])
                        keng = nc.vector if dve_k else nc.gpsimd
                        keng.tensor_copy(aug_k[0:64, j0:j0 + QB],
                                         aug_q[0:64, j0:j0 + QB])

                    HH = HNT // 2      # chunks per hp tile (PSUM bank limit)

                    def u_hmk(sub):
                        # separate PSUM tile per 4 chunks: a matmul output
                        # must not cross a 512-f32 PSUM bank boundary
                        st[f'hp{sub}'] = pso.tile([128, HH * hw], F32,
                                                  name=f"hp_{tag}_{half}_{sub}",
                                                  tag="ot")
                        for k in range(HH):
                            hmk_thunk(st[f'hp{sub}'][:, k * hw:(k + 1) * hw],
                                      half * HNT + sub * HH + k)

                    def u_hcopy():
                        for sub in range(2):
                            hp = st[f'hp{sub}']
                            h0 = (half * HNT + sub * HH) * CHW
                            if full65:
                                src = bass.AP(tensor=hp.tensor, offset=hp.offset,
                                              ap=[hp.ap[0], [hw, HH], [1, CHW]])
                                nc.vector.tensor_copy(
                                    H_aug[:, h0:h0 + HH * CHW], src)
                            else:
                                dst = bass.AP(tensor=H_aug.tensor,
                                              offset=H_aug.offset + h0,
                                              ap=[H_aug.ap[0], [CHW, HH], [1, 64]])
                                nc.vector.tensor_copy(dst, hp)
                                ones_col = bass.AP(
                                    tensor=H_aug.tensor,
                                    offset=H_aug.offset + h0 + 64,
                                    ap=[H_aug.ap[0], [CHW, HH]])
                                nc.vector.memset(ones_col, 1.0)
                        if host_r2_row is None:
                            st['negr2'] = small.tile([128, HNT], F32,
                                                     name=f"nr_{tag}_{half}",
                                                     tag="negr2")
                            st['scr'] = small.tile([128, 64], F16,
                                                   name=f"scr_{tag}_{half}",
                                                   tag="scr")

                    def u_r2(k2):
                        for k in (2 * k2, 2 * k2 + 1):
                            mc = half * HNT + k
                            nc.vector.tensor_mul(
                                st['scr'], H_aug[:, mc * CHW:mc * CHW + 64],
                                H_aug[:, mc * CHW:mc * CHW + 64])
                            nc.vector.tensor_reduce(
                                st['negr2'][:, k:k + 1], st['scr'],
                                axis=AX.X, op=ALU.add)

                    def u_tail():
                        negr2h = small.tile([128, HNT], F16,
                                            name=f"nrh_{tag}_{half}",
                                            tag="negr2h")
                        nc.vector.tensor_scalar_mul(negr2h, st['negr2'], -1.0)
                        ntp = pso.tile([HNT, 128], F16,
                                       name=f"ntp_{tag}_{half}", tag="ot")
                        nc.tensor.transpose(ntp, negr2h, idf16[:128, :128])
                        nrsb = small.tile([HNT, 128], F16,
                                          name=f"nrsb_{tag}_{half}", tag="nrsb")
                        nc.vector.tensor_copy(nrsb, ntp)
                        r64 = aug_q[64:65, :]
                        r64v = bass.AP(tensor=r64.tensor,
                                       offset=r64.offset + j0,
                                       ap=[r64.ap[0], [128, HNT], [1, 128]])
                        (r64_eng or nc.sync).dma_start(r64v, nrsb)

                    us = [u_proj, lambda: u_hmk(0), lambda: u_hmk(1), u_hcopy]
                    if host_r2_row is None:
                        us += [lambda k2=k2: u_r2(k2) for k2 in range(HNT // 2)]
                        us.append(u_tail)
                    return us

                return (aug_q, aug_k, H_aug), half_units

            # ---- attention core ----------------------------------------------
            # PV(mc) is emitted AFTER scores(mc+1): the PE queue is in-order,
            # so this keeps the next chunk's scores flowing while exp/mask of
            # the current chunk complete (PV parks in the wait queue).
            def attention(aug_q, aug_k, H_aug, out_cb, tag,
                          feed=None, feed_start=0, delay_cb0=False):
                """feed: list of thunks emitted one-per-chunk starting at
                global chunk index feed_start — lets the next stage's build
                work interleave between this pass's mask-mults without
                overflowing the engines' 4-deep wait queues."""
                feed = list(feed) if feed else []
                cb0_args = None
                for qb in range(NQB):
                    ot = pso.tile([65, QB], F32, name=f"ot_{tag}_{qb}", tag="ot")

                    def emit_pv(pmm, mc, ot=ot):
                        for nb in range(QB // 512):
                            nc.tensor.matmul(
                                ot[:, nb * 512:(nb + 1) * 512],
                                H_aug[:, mc * CHW:mc * CHW + 65],
                                pmm[:, nb * 512:(nb + 1) * 512],
                                start=(mc == 0), stop=(mc == NT - 1))

                    # PV lag 2: the critical cycle exp(k)->mask(k)->PV(k)->
                    # [PE in-order]->scores->exp then spans 3 chunks instead
                    # of 2, dropping the steady-state cadence to ~max(engine).
                    pend = []
                    for mc in range(NT):
                        sc = pss.tile([128, QB], F32,
                                      name=f"sc_{tag}_{qb}_{mc}", tag="sc")
                        for nb in range(QB // 512):
                            q0 = qb * QB + nb * 512
                            nc.tensor.matmul(
                                sc[:, nb * 512:(nb + 1) * 512],
                                aug_k[:, mc * 128:(mc + 1) * 128],
                                aug_q[:, q0:q0 + 512],
                                start=True, stop=True)
                        if len(pend) >= 2:
                            emit_pv(*pend.pop(0))
                        pm = ppool.tile([128, QB], BF16,
                                        name=f"pm_{tag}_{qb}_{mc}", tag="pm",
                                        bufs=6)
                        nc.scalar.activation(pm, sc, AF.Exp)
                        pmm = ppool.tile([128, QB], BF16,
                                         name=f"pmm_{tag}_{qb}_{mc}", tag="pmm")
                        nc.vector.tensor_mul(
                            pmm, pm,
                            mask_sb[:, mc * N + qb * QB: mc * N + qb * QB + QB])
                        pend.append((pmm, mc))
                        if feed and qb * NT + mc >= feed_start:
                            feed.pop(0)()
                    for pv in pend:
                        emit_pv(*pv)
                    if qb == 0 and delay_cb0:
                        cb0_args = (qb, ot)
                    else:
                        out_cb(qb, ot)
                for th in feed:
                    th()
                if cb0_args is not None:
                    out_cb(*cb0_args)

            # ---- L1 prologue: head 0 aug only (heads 1-3 interleave into
            # the passes so their DVE work overlaps attention) ------------------
            catT = [const.tile([128, N], F32, name=f"catT{t}") for t in range(HD // 128)]

            def l1_proj(half, h):
                j0 = half * QB
                ps = pso.tile([65, QB], F32, name=f"prj_{h}_{half}", tag="ot")
                for nb in range(QB // 512):
                    nc.tensor.matmul(ps[:, nb * 512:(nb + 1) * 512],
                                     whT_sb[:, h * 66:h * 66 + 65],
                                     xT[:, j0 + nb * 512:j0 + (nb + 1) * 512],
                                     start=True, stop=True)
                return ps

            augs = [make_aug(f"l1h{h}", full65=True,
                             host_r2_row=negr2_d[h:h + 1, :],
                             dve_k=(h == 0)) for h in range(H)]

            def head_units(h):
                _, half_units = augs[h]

                def hmk(dst, mc, h=h):
                    nc.tensor.matmul(dst, xT[:, mc * 128:(mc + 1) * 128],
                                     whT_sb[:, h * 66:(h + 1) * 66],
                                     start=True, stop=True)

                us = []
                for half in range(NQB):
                    us += half_units(half,
                                     lambda half=half, h=h: l1_proj(half, h),
                                     hmk)
                return us

            for u in head_units(0):
                u()
            for mc in range(2, NT):
                mask_dma(mc)

            # ---- L1 passes ----------------------------------------------------
            def make_l1_cb(h):
                def l1_cb(qb, ot):
                    # one fast DVE copy releases the PSUM accumulator early —
                    # holding it through the DMA roundtrip blocks the next
                    # stage's PSUM tiles in the pool ring.
                    o1 = work.tile([65, QB], F32, name=f"o1_{h}_{qb}", tag="o1")
                    nc.vector.tensor_copy(o1, ot)
                    # denominators: row -> DRAM -> [128, QB/128] gather so the
                    # reciprocal runs on all lanes, then broadcast back.
                    rd = drb.tile([1, QB], F32, name=f"rd_{h}_{qb}", tag="rd")
                    nc.sync.dma_start(rd, o1[64:65, :])
                    dn = small.tile([128, QB // 128], F32, name=f"dn_{h}_{qb}",
                                    tag="dn")
                    nc.sync.dma_start(dn, rd.rearrange("o (c p) -> p (o c)", p=128))
                    rc = small.tile([128, QB // 128], F32, name=f"rc_{h}_{qb}",
                                    tag="rc")
                    nc.vector.reciprocal(rc, dn)
                    rd2 = drb.tile([1, QB], F32, name=f"rd2_{h}_{qb}", tag="rd2")
                    nc.sync.dma_start(rd2.rearrange("o (c p) -> p (o c)", p=128), rc)
                    recb = work.tile([64, QB], F32, name=f"rcb_{h}_{qb}", tag="rcb")
                    nc.sync.dma_start(recb, rd2.to_broadcast([64, QB]))
                    dst = catT[h // 2][(h % 2) * 64:(h % 2) * 64 + 64,
                                      qb * QB:(qb + 1) * QB]
                    nc.vector.tensor_mul(dst, o1[0:64, :], recb)
                    if use_bh:
                        nc.vector.tensor_scalar_add(
                            dst, dst, bh_cols[(h % 2) * 64:(h % 2) * 64 + 64,
                                              h // 2:h // 2 + 1])
                return l1_cb

            # ---- L2 build pieces (leaky + projection + aug), thunk-granular --
            zT = [const.tile([128, N], F32R, name=f"zT{t}") for t in range(HD // 128)]
            (aug_q2, aug_k2, H_aug2), half_units2 = make_aug(
                "l2", full65=False, r64_eng=nc.scalar)

            def l2_proj(half):
                j0 = half * QB
                ps = pso.tile([65, QB], F32, name=f"prj2_{half}", tag="ot")
                for nb in range(QB // 512):
                    for kc in range(2):
                        nc.tensor.matmul(
                            ps[0:64, nb * 512:(nb + 1) * 512],
                            woT_sb[:, kc * D:(kc + 1) * D],
                            zT[kc][:, j0 + nb * 512:j0 + (nb + 1) * 512],
                            start=(kc == 0), stop=(kc == 1))
                return ps

            def l2_hmk(dst, mc):
                for kc in range(2):
                    nc.tensor.matmul(dst, zT[kc][:, mc * 128:(mc + 1) * 128],
                                     woT_sb[:, kc * D:(kc + 1) * D],
                                     start=(kc == 0), stop=(kc == 1))

            def l2_units(half):
                us = []
                for part in (2 * half, 2 * half + 1):
                    def u_leak(part=part):
                        j0 = part * 512
                        for t in range(HD // 128):
                            nc.vector.scalar_tensor_tensor(
                                zT[t][:, j0:j0 + 512], catT[t][:, j0:j0 + 512],
                                0.2, catT[t][:, j0:j0 + 512],
                                op0=ALU.mult, op1=ALU.max)
                    us.append(u_leak)
                us += half_units2(half, lambda half=half: l2_proj(half), l2_hmk)
                return us

            # ---- L1 passes: head h+1's aug (or the first L2 half) feeds in
            # one thunk per chunk so its PE/DVE work hides under attention.
            for h in range(H):
                (aug_q, aug_k, H_aug), _ = augs[h]
                if h < H - 1:
                    feed, fs = head_units(h + 1), 0
                else:
                    feed, fs = l2_units(0), 3 * NT // 2
                attention(aug_q, aug_k, H_aug, make_l1_cb(h), f"l1h{h}",
                          feed=feed, feed_start=fs)

            # ---- L2 + epilogue ------------------------------------------------
            def l2_cb(qb, ot):
                NB = QB // 128
                # bounce the accumulator to SBUF (transpose input must be
                # SBUF; GPSIMD can't read PSUM so this rides DVE)
                o1 = work.tile([65, QB], F32, name=f"l2o1_{qb}", tag="o1")
                nc.vector.tensor_copy(o1, ot)
                # numerators AND denominator row -> [q, d|denom] via PE
                # transposes; per-block reciprocal straight off col 64 (no
                # DMA roundtrip needed in this layout).
                TW = 65
                tpb = work.tile([128, NB * TW], F32, name=f"tpb_{qb}", tag="tpb")
                ctr = work.tile([128, NB * D], F32, name=f"ctr_{qb}", tag="ctr")
                vars_ = small.tile([128, NB], F32, name=f"vars_{qb}", tag="vars")
                vscr = small.tile([128, 64], F32, name=f"vscr_{qb}", tag="vscr")
                for j in range(NB):
                    ftp = pss.tile([128, TW], F32, name=f"ftp_{qb}_{j}", tag="sc")
                    nc.tensor.transpose(ftp, o1[0:65, j * 128:(j + 1) * 128],
                                        idf32[:65, :65])
                    nc.vector.tensor_copy(tpb[:, j * TW:(j + 1) * TW], ftp)
                    rcj = small.tile([128, 1], F32, name=f"rcj_{qb}_{j}",
                                     tag="rcj")
                    nc.vector.reciprocal(rcj, tpb[:, j * TW + 64:(j + 1) * TW])
                    o2n = small.tile([128, D], F32, name=f"o2n_{qb}_{j}", tag="fo2")
                    nc.vector.tensor_scalar(o2n, tpb[:, j * TW:j * TW + D],
                                            rcj, None, op0=ALU.mult)
                    if use_bo:
                        nc.vector.tensor_add(o2n, o2n, bo_row)
                    z2 = small.tile([128, D], F32, name=f"z2_{qb}_{j}", tag="fz2")
                    nc.vector.scalar_tensor_tensor(z2, o2n, 0.2, o2n,
                                                   op0=ALU.mult, op1=ALU.max)
                    s1 = small.tile([128, 1], F32, name=f"s1_{qb}_{j}", tag="fs1")
                    nc.vector.tensor_reduce(s1, z2, axis=AX.X, op=ALU.add)
                    mu = small.tile([128, 1], F32, name=f"mu_{qb}_{j}", tag="fmu")
                    nc.vector.tensor_scalar_mul(mu, s1, 1.0 / D)
                    cj = ctr[:, j * D:(j + 1) * D]
                    nc.vector.tensor_scalar(cj, z2, mu, None, op0=ALU.subtract)
                    nc.vector.tensor_mul(vscr, cj, cj)
                    nc.vector.tensor_reduce(vars_[:, j:j + 1], vscr,
                                            axis=AX.X, op=ALU.add)
                # vars_ holds sum(ctr^2) = D*var; rstd*sqrt(D) folds the 1/D
                # back in: out = ctr * sqrt(D)/sqrt(sumsq + D*eps).
                # rsqrt via exp(-0.5*ln(x)) keeps ACT in the exp/ln table set
                # (a Sqrt would force two ACT_TABLE_LOADs mid exp-stream).
                lnv = small.tile([128, NB], F32, name=f"lnv_{qb}", tag="stds")
                nc.scalar.activation(lnv, vars_, AF.Ln, bias=eps_col)
                rstds = small.tile([128, NB], F32, name=f"rstds_{qb}", tag="rstds")
                nc.scalar.activation(rstds, lnv, AF.Exp, scale=-0.5)
                on = work.tile([128, NB * D], F32, name=f"on_{qb}", tag="on")
                for j in range(NB):
                    oj = on[:, j * D:(j + 1) * D]
                    nc.vector.tensor_scalar(oj, ctr[:, j * D:(j + 1) * D],
                                            rstds[:, j:j + 1], float(np.sqrt(D)),
                                            op0=ALU.mult, op1=ALU.mult)
                    if use_gamma:
                        nc.vector.tensor_mul(oj, oj, gamma_row)
                    if use_beta:
                        nc.vector.tensor_add(oj, oj, beta_row)
                # one DMA per qb: element (p, j, d) -> out[qb*QB + j*128 + p, d]
                odst = bass.AP(tensor=out_d, offset=qb * QB * D,
                               ap=[[D, 128], [128 * D, NB], [1, D]])
                nc.sync.dma_start(odst, on)

            attention(aug_q2, aug_k2, H_aug2, l2_cb, "l2",
                      feed=l2_units(1), feed_start=0)

    return nc


# ---------------------------------------------------------------------------
# Host-side runner (cached compiled executable via bass2jax/PJRT)
# ---------------------------------------------------------------------------
_RUNNER_CACHE = {}


def _make_runner(nc, n_cores):
    import jax
    from jax.sharding import Mesh, PartitionSpec
    from jax.experimental.shard_map import shard_map
    from concourse import bass2jax
    from concourse.bass2jax import _bass_exec_p, install_neuronx_cc_hook

    install_neuronx_cc_hook()
    partition_name = nc.partition_id_tensor.name if nc.partition_id_tensor else None

    in_names, out_names, out_avals = [], [], []
    for alloc in nc.m.functions[0].allocations:
        if not isinstance(alloc, mybir.MemoryLocationSet):
            continue
        name = alloc.memorylocations[0].name
        if alloc.kind == "ExternalInput":
            if name != partition_name:
                in_names.append(name)
        elif alloc.kind == "ExternalOutput":
            out_names.append(name)
            out_avals.append(jax.core.ShapedArray(tuple(alloc.tensor_shape),
                                                  mybir.dt.np(alloc.dtype)))
    n_params = len(in_names)
    n_outs = len(out_avals)
    all_in_names = list(in_names) + list(out_names)
    if partition_name is not None:
        all_in_names.append(partition_name)

    def _body(*args):
        operands = list(args)
        if partition_name is not None:
            operands.append(bass2jax.partition_id_tensor())
        outs = _bass_exec_p.bind(
            *operands,
            out_avals=tuple(out_avals),
            in_names=tuple(all_in_names),
            out_names=tuple(out_names),
            lowering_input_output_aliases=(),
            sim_require_finite=True,
            sim_require_nnan=True,
            nc=nc,
        )
        return tuple(outs)

    donate = tuple(range(n_params, n_params + n_outs))

    if n_cores == 1:
        jitted = jax.jit(_body, donate_argnums=donate, keep_unused=True)

        def run(in_maps):
            args = [np.asarray(in_maps[0][n]) for n in in_names]
            zeros = [np.zeros(a.shape, a.dtype) for a in out_avals]
            outs = jitted(*args, *zeros)
            jax.block_until_ready(outs)
            return [{n: np.asarray(outs[i]) for i, n in enumerate(out_names)}]

        return run

    devices = jax.devices()[:n_cores]
    mesh = Mesh(np.asarray(devices), ("core",))
    in_specs = (PartitionSpec("core"),) * (n_params + n_outs)
    out_specs = (PartitionSpec("core"),) * n_outs
    jitted = jax.jit(
        shard_map(_body, mesh=mesh, in_specs=in_specs, out_specs=out_specs,
                  check_rep=False),
        donate_argnums=donate,
        keep_unused=True,
    )

    def run(in_maps):
        per_core = [[np.asarray(m[n]) for n in in_names] for m in in_maps]
        concat_in = [np.concatenate([per_core[c][i] for c in range(n_cores)], axis=0)
                     for i in range(n_params)]
        concat_zero = [np.zeros((a.shape[0] * n_cores,) + a.shape[1:], a.dtype)
                       for a in out_avals]
        outs = jitted(*concat_in, *concat_zero)
        jax.block_until_ready(outs)
        results = []
        for c in range(n_cores):
            d = {}
            for i, n in enumerate(out_names):
                per_len = out_avals[i].shape[0]
                d[n] = np.asarray(outs[i][c * per_len:(c + 1) * per_len])
            results.append(d)
        return results

    return run


def _get_runner(flags, n_cores):
    key = (flags, n_cores)
    if key not in _RUNNER_CACHE:
        nc = build_gat(use_bh=flags[0], use_bo=flags[1],
                       use_gamma=flags[2], use_beta=flags[3])
        _RUNNER_CACHE[key] = (_make_runner(nc, n_cores), nc)
    return _RUNNER_CACHE[key][0]


def make_in_maps(x, graph, Wh, bh, Wo, bo, gamma, beta):
    B, N, C = x.shape
    H, D, _ = Wh.shape
    flags = (bool(np.any(bh)), bool(np.any(bo)),
             bool(np.any(gamma != 1.0)), bool(np.any(beta)))
    mask = (graph + np.eye(N, dtype=graph.dtype)) > 0
    maskt = np.ascontiguousarray(mask.T).astype(ml_dtypes.bfloat16)
    # augmented projection weights: per head 65 output cols; col 64 selects
    # the ones row of x^T so the proj PSUM carries the ones row for free.
    wht = np.zeros((C + 1, H * 66), np.float32)
    for h in range(H):
        wht[:C, h * 66:h * 66 + D] = Wh[h].T            # [c, d]
        wht[C, h * 66 + 64] = 1.0
    # woT_sb[p, kc*D+d] = Wo[d, kc*128+p]
    wot = np.ascontiguousarray(
        Wo.T.reshape(2, 128, D).transpose(1, 0, 2).reshape(128, 2 * D)).astype(np.float32)
    in_maps = []
    for b in range(B):
        xa = np.concatenate([x[b].T, np.ones((1, N), np.float32)], axis=0)
        hb = np.einsum('nc,hdc->hnd', x[b], Wh)            # [H, N, D]
        negr2 = -np.square(hb).sum(-1)                      # [H, N]
        m = {"xt": np.ascontiguousarray(xa).astype(np.float32),
             "maskt": maskt, "wht": wht, "wot": wot,
             "negr2": negr2.astype(np.float16)}
        if flags[0]:
            m["bh"] = np.ascontiguousarray(
                np.asarray(bh, np.float32).reshape(-1).reshape(2, 128).T)
        if flags[1]:
            m["bo"] = np.asarray(bo, np.float32)
        if flags[2]:
            m["gamma"] = np.asarray(gamma, np.float32)
        if flags[3]:
            m["beta"] = np.asarray(beta, np.float32)
        in_maps.append(m)
    return in_maps, flags


def kernel(x, graph, Wh, bh, Wo, bo, gamma, beta):
    x = np.asarray(x)
    B = x.shape[0]
    in_maps, flags = make_in_maps(np.asarray(x, np.float32), np.asarray(graph),
                                  np.asarray(Wh, np.float32),
                                  np.asarray(bh, np.float32),
                                  np.asarray(Wo, np.float32),
                                  np.asarray(bo, np.float32),
                                  np.asarray(gamma, np.float32),
                                  np.asarray(beta, np.float32))
    run = _get_runner(flags, B)
    results = run(in_maps)
    return np.stack([r["out"] for r in results], axis=0)


# revision 67
# speedup vs baseline: 1.1507x; 1.1507x over previous
"""GAT spatial kernel for trn2 (nn_GATSpatial_36112085025002) — v2.

Strategy
--------
Data-parallel over B=8 across the 8 NeuronCores; each core runs the full
2-layer GAT for one batch element.

v2 design (vs v1 baseline at ~455us):
  - ACT engine does ONLY the exp over the N^2 scores (the hard floor);
    all copies move to DVE/Pool/DMA.
  - Scores in transposed layout sT[keys, q] via K=65 augmented contraction:
    stationary aug_k rows 0-63 = h^T fp16, row 64 = ones (produced for free
    by an augmented projection: x^T gets a ones row, Wh^T gets a selector
    column); moving aug_q rows 0-63 = h^T, row 64 = -||h_q||^2 (softmax
    shift; exact by shift-invariance, so fp16 precision is fine).
  - -r^2 per chunk via one fused tensor_tensor_reduce off the PV-stationary
    tile, then PE-transpose + one strided DMA into aug_q row 64.
  - H_aug (PV stationary, [128, 65] per chunk incl ones col for free
    denominators) built with DMA-xbar transposes — zero PSUM traffic, so
    the whole L1 aug prep runs in a prologue overlapped with the mask DMA
    and the attention passes run back-to-back with no PE gaps (keeps the
    PE p-state ramped).
  - exp: ACT [128,1024] PSUM->bf16; mask as bf16 multiply on DVE (2x mode).
  - PSUM: scores [128,1024]x2 + out accum [65,1024]x2 = exactly 8 banks.
  - L1 normalization: denominators DMA-roundtrip (DRAM gather -> 128-lane
    reciprocal -> broadcast), numerators scaled straight out of PSUM on DVE.
  - L2 epilogue: numerators -> bf16 -> DMA-xbar transpose to [q, d], per-
    block reciprocal from a gathered [128, 8] column layout, leaky+LN on
    DVE with batched sqrt on ACT.
"""
import sys

sys.path.insert(0, '/opt/trn_rl_repo')

import numpy as np
import ml_dtypes

import concourse.bass as bass
import concourse.tile as tile
import concourse.mybir as mybir
from concourse.masks import make_identity

F32 = mybir.dt.float32
F32R = mybir.dt.float32r
F16 = mybir.dt.float16
BF16 = mybir.dt.bfloat16
AF = mybir.ActivationFunctionType
ALU = mybir.AluOpType
AX = mybir.AxisListType

N_CORES = 8
LN_EPS = 1e-5

# ---------------------------------------------------------------------------
# walrus workaround: this compiler build rejects >1 sync-wait per instruction.
# Split extra waits into standalone EventSemaphore instructions.
# ---------------------------------------------------------------------------
_orig_commit = tile.TileContext._commit_and_lower


def _patched_commit(self, inst, *args, **kwargs):
    si = getattr(inst, "sync_info", None)
    waits = list(si.on_wait) if si is not None and si.on_wait else []
    if len(waits) > 1:
        for w in waits[:-1]:
            ev = mybir.InstEventSemaphore(
                name=self.nc.get_next_instruction_name(),
                engine=inst.engine,
                ins=[],
                outs=[],
                sync_info=mybir.SyncInfo(on_wait=[w], on_update=[]),
            )
            _orig_commit(self, ev, *args, **kwargs)
        si.on_wait = [waits[-1]]
        inst.sync_info = si
    return _orig_commit(self, inst, *args, **kwargs)


def _patched_drain_and_barrier(self, tick_clock, wait_clock):
    from concourse.tile import ScopedClock

    nc = self.nc
    dummy = mybir.InstDrain(
        name="tail-drain-waits", ins=[], outs=[], bass_is_fusable=False
    )
    dummy.engine = nc.sync.engine
    wait_clock.add_sem_waits(dummy, ScopedClock({None: tick_clock.global_clock}))
    waits = list(dummy.sync_info.on_wait) if dummy.sync_info else []
    for w in waits:
        ev = mybir.InstEventSemaphore(
            name=nc.get_next_instruction_name(),
            engine=nc.sync.engine,
            ins=[],
            outs=[],
            sync_info=mybir.SyncInfo(on_wait=[w], on_update=[]),
        )
        nc.sync.add_instruction(ev)
    nc.sync.drain()

    nc.all_engine_barrier()
    assert self.sems is not None
    popped = nc._tile_sem_poison_stack.pop()
    assert popped is self._sem_poison
    nc.clear_and_free_semaphores(list(self.sems.allocated().values()))
    nc.all_engine_barrier()


if getattr(tile.TileContext, "_wait_split_patched", False) is False:
    tile.TileContext._commit_and_lower = _patched_commit
    tile.TileContext._drain_and_barrier = _patched_drain_and_barrier
    tile.TileContext._wait_split_patched = True


# ---------------------------------------------------------------------------
# Kernel builder
# ---------------------------------------------------------------------------
def build_gat(N=2048, C=64, H=4, D=64,
              use_bh=False, use_bo=False, use_gamma=False, use_beta=False):
    assert N % 128 == 0
    NT = N // 128                     # key chunks
    QB = 1024                         # q block
    NQB = N // QB
    HD = H * D
    CHW = 65                          # H_aug per-chunk column stride (64 + ones)

    nc = bass.Bass(trn_type="TRN2")
    xt_d = nc.dram_tensor("xt", [C + 1, N], F32R, kind="ExternalInput")
    maskt_d = nc.dram_tensor("maskt", [N, N], BF16, kind="ExternalInput")
    wht_d = nc.dram_tensor("wht", [C + 1, H * 66], F32R, kind="ExternalInput")
    negr2_d = nc.dram_tensor("negr2", [H, N], F16, kind="ExternalInput")
    wot_d = nc.dram_tensor("wot", [128, (HD // 128) * D], F32R, kind="ExternalInput")
    bh_d = nc.dram_tensor("bh", [128, HD // 128], F32, kind="ExternalInput") if use_bh else None
    bo_d = nc.dram_tensor("bo", [D], F32, kind="ExternalInput") if use_bo else None
    gamma_d = nc.dram_tensor("gamma", [D], F32, kind="ExternalInput") if use_gamma else None
    beta_d = nc.dram_tensor("beta", [D], F32, kind="ExternalInput") if use_beta else None
    out_d = nc.dram_tensor("out", [N, D], F32, kind="ExternalOutput")

    with tile.TileContext(nc) as tc:
        import contextlib
        ctx = contextlib.ExitStack()
        with ctx:
            const = ctx.enter_context(tc.tile_pool(name="const", bufs=1))
            aug = ctx.enter_context(tc.tile_pool(name="aug", bufs=3))
            work = ctx.enter_context(tc.tile_pool(name="work", bufs=2))
            small = ctx.enter_context(tc.tile_pool(name="small", bufs=4))
            ppool = ctx.enter_context(tc.tile_pool(name="ppool", bufs=4))
            pss = ctx.enter_context(tc.tile_pool(name="pss", bufs=2, space="PSUM"))
            drb = ctx.enter_context(tc.tile_pool(name="drb", bufs=4, space="DRAM"))
            pso = ctx.enter_context(tc.tile_pool(name="pso", bufs=2, space="PSUM"))

            # ---- constants ----------------------------------------------------
            idf32 = const.tile([128, 128], F32, name="idf32")
            make_identity(nc, idf32)
            idf16 = const.tile([128, 128], F16, name="idf16")
            nc.vector.tensor_copy(idf16, idf32)
            eps_col = const.tile([128, 1], F32, name="eps_col")
            nc.vector.memset(eps_col, LN_EPS * D)
            onesrow = const.tile([1, N], F16, name="onesrow")
            nc.gpsimd.memset(onesrow, 1.0)

            xT = const.tile([C + 1, N], F32R, name="xT")
            nc.sync.dma_start(xT, xt_d[:, :])
            whT_sb = const.tile([C + 1, H * 66], F32R, name="whT_sb")
            nc.sync.dma_start(whT_sb, wht_d[:, :])
            woT_sb = const.tile([128, 2 * D], F32R, name="woT_sb")
            nc.sync.dma_start(woT_sb, wot_d[:, :])

            # mask resident in SBUF: [128, NT*N] bf16, chunk mc at cols
            # [mc*N, (mc+1)*N); one DMA per chunk, split across both hwdge
            # queues (SP + ACT) in consumption order. Chunks 2+ are emitted
            # after the head-0 aug build so its r64 DMA isn't queued behind
            # them on SP (see below).
            mask_sb = const.tile([128, NT * N], BF16, name="mask_sb")

            def mask_dma(mc):
                eng = nc.sync if mc % 2 == 0 else nc.scalar
                eng.dma_start(mask_sb[:, mc * N:(mc + 1) * N],
                              maskt_d[mc * 128:(mc + 1) * 128, :])
            mask_dma(0)
            mask_dma(1)

            bh_cols = None
            if use_bh:
                bh_cols = const.tile([128, 2], F32, name="bh_cols")
                nc.sync.dma_start(bh_cols, bh_d[:, :])
            bo_row = gamma_row = beta_row = None
            if use_bo:
                bo_row = const.tile([128, D], F32, name="bo_row")
                nc.sync.dma_start(bo_row, bo_d.to_broadcast([128, D]))
            if use_gamma:
                gamma_row = const.tile([128, D], F32, name="gamma_row")
                nc.sync.dma_start(gamma_row, gamma_d.to_broadcast([128, D]))
            if use_beta:
                beta_row = const.tile([128, D], F32, name="beta_row")
                nc.sync.dma_start(beta_row, beta_d.to_broadcast([128, D]))

            # ---- aug builder --------------------------------------------------
            HNT = NT // 2             # chunks per half

            def make_aug(tag, full65, r64_eng=None, host_r2_row=None,
                         dve_k=False):
                """Returns ((aug_q, aug_k, H_aug), half_units). half_units(
                half, proj_thunk, hmk_thunk) -> list of small thunks building
                columns [half*QB,(half+1)*QB): d-major proj -> aug copies;
                key-major H chunks straight from mini-matmuls (no transposes);
                fused -r^2 per chunk; strided DMA into aug_q row 64. Thunk-
                granular so the work interleaves into a running pass."""
                aug_q = aug.tile([65, N], F16, name=f"aq_{tag}", tag="aug_q")
                aug_k = aug.tile([65, N], F16, name=f"ak_{tag}", tag="aug_k")
                H_aug = aug.tile([128, CHW * NT], F16, name=f"Ha_{tag}", tag="H_aug")
                rows = 65 if full65 else 64
                hw = 66 if full65 else 64   # k-major matmul output width
                # (66: f32r moving operands need an even free size; col 64 is
                # the ones column, col 65 zero padding)

                def half_units(half, proj_thunk, hmk_thunk):
                    j0 = half * QB
                    st = {}

                    def u_proj():
                        if half == 0:
                            if host_r2_row is not None:
                                # host-precomputed -r^2 straight into row 64
                                nc.sync.dma_start(aug_q[64:65, :], host_r2_row)
                            nc.sync.dma_start(aug_k[64:65, :], onesrow)
                        ps = proj_thunk()
                        # GPSIMD can't read PSUM: aug_q from PSUM on DVE,
                        # aug_k mirrored from aug_q on Pool (SBUF->SBUF).
                        nc.vector.tensor_copy(aug_q[0:64, j0:j0 + QB],
                                              ps[0:64, :])
                        keng = nc.vector if dve_k else nc.gpsimd
                        keng.tensor_copy(aug_k[0:64, j0:j0 + QB],
                                         aug_q[0:64, j0:j0 + QB])

                    HH = HNT // 2      # chunks per hp tile (PSUM bank limit)

                    def u_hmk(sub):
                        # separate PSUM tile per 4 chunks: a matmul output
                        # must not cross a 512-f32 PSUM bank boundary
                        st[f'hp{sub}'] = pso.tile([128, HH * hw], F32,
                                                  name=f"hp_{tag}_{half}_{sub}",
                                                  tag="ot")
                        for k in range(HH):
                            hmk_thunk(st[f'hp{sub}'][:, k * hw:(k + 1) * hw],
                                      half * HNT + sub * HH + k)

                    def u_hcopy():
                        for sub in range(2):
                            hp = st[f'hp{sub}']
                            h0 = (half * HNT + sub * HH) * CHW
                            if full65:
                                src = bass.AP(tensor=hp.tensor, offset=hp.offset,
                                              ap=[hp.ap[0], [hw, HH], [1, CHW]])
                                nc.vector.tensor_copy(
                                    H_aug[:, h0:h0 + HH * CHW], src)
                            else:
                                dst = bass.AP(tensor=H_aug.tensor,
                                              offset=H_aug.offset + h0,
                                              ap=[H_aug.ap[0], [CHW, HH], [1, 64]])
                                nc.vector.tensor_copy(dst, hp)
                                ones_col = bass.AP(
                                    tensor=H_aug.tensor,
                                    offset=H_aug.offset + h0 + 64,
                                    ap=[H_aug.ap[0], [CHW, HH]])
                                nc.vector.memset(ones_col, 1.0)
                        if host_r2_row is None:
                            st['negr2'] = small.tile([128, HNT], F32,
                                                     name=f"nr_{tag}_{half}",
                                                     tag="negr2")
                            st['scr'] = small.tile([128, 64], F16,
                                                   name=f"scr_{tag}_{half}",
                                                   tag="scr")

                    def u_r2(k2):
                        for k in (2 * k2, 2 * k2 + 1):
                            mc = half * HNT + k
                            nc.vector.tensor_mul(
                                st['scr'], H_aug[:, mc * CHW:mc * CHW + 64],
                                H_aug[:, mc * CHW:mc * CHW + 64])
                            nc.vector.tensor_reduce(
                                st['negr2'][:, k:k + 1], st['scr'],
                                axis=AX.X, op=ALU.add)

                    def u_tail():
                        negr2h = small.tile([128, HNT], F16,
                                            name=f"nrh_{tag}_{half}",
                                            tag="negr2h")
                        nc.vector.tensor_scalar_mul(negr2h, st['negr2'], -1.0)
                        ntp = pso.tile([HNT, 128], F16,
                                       name=f"ntp_{tag}_{half}", tag="ot")
                        nc.tensor.transpose(ntp, negr2h, idf16[:128, :128])
                        nrsb = small.tile([HNT, 128], F16,
                                          name=f"nrsb_{tag}_{half}", tag="nrsb")
                        nc.vector.tensor_copy(nrsb, ntp)
                        r64 = aug_q[64:65, :]
                        r64v = bass.AP(tensor=r64.tensor,
                                       offset=r64.offset + j0,
                                       ap=[r64.ap[0], [128, HNT], [1, 128]])
                        (r64_eng or nc.sync).dma_start(r64v, nrsb)

                    us = [u_proj, lambda: u_hmk(0), lambda: u_hmk(1), u_hcopy]
                    if host_r2_row is None:
                        us += [lambda k2=k2: u_r2(k2) for k2 in range(HNT // 2)]
                        us.append(u_tail)
                    return us

                return (aug_q, aug_k, H_aug), half_units

            # ---- attention core ----------------------------------------------
            # PV(mc) is emitted AFTER scores(mc+1): the PE queue is in-order,
            # so this keeps the next chunk's scores flowing while exp/mask of
            # the current chunk complete (PV parks in the wait queue).
            def attention(aug_q, aug_k, H_aug, out_cb, tag,
                          feed=None, feed_start=0, delay_cb0=False):
                """feed: list of thunks emitted one-per-chunk starting at
                global chunk index feed_start — lets the next stage's build
                work interleave between this pass's mask-mults without
                overflowing the engines' 4-deep wait queues."""
                feed = list(feed) if feed else []
                cb0_args = None
                for qb in range(NQB):
                    ot = pso.tile([65, QB], F32, name=f"ot_{tag}_{qb}", tag="ot")

                    def emit_pv(pmm, mc, ot=ot):
                        for nb in range(QB // 512):
                            nc.tensor.matmul(
                                ot[:, nb * 512:(nb + 1) * 512],
                                H_aug[:, mc * CHW:mc * CHW + 65],
                                pmm[:, nb * 512:(nb + 1) * 512],
                                start=(mc == 0), stop=(mc == NT - 1))

                    # PV lag 2: the critical cycle exp(k)->mask(k)->PV(k)->
                    # [PE in-order]->scores->exp then spans 3 chunks instead
                    # of 2, dropping the steady-state cadence to ~max(engine).
                    pend = []
                    for mc in range(NT):
                        sc = pss.tile([128, QB], F32,
                                      name=f"sc_{tag}_{qb}_{mc}", tag="sc")
                        for nb in range(QB // 512):
                            q0 = qb * QB + nb * 512
                            nc.tensor.matmul(
                                sc[:, nb * 512:(nb + 1) * 512],
                                aug_k[:, mc * 128:(mc + 1) * 128],
                                aug_q[:, q0:q0 + 512],
                                start=True, stop=True)
                        if len(pend) >= 1:
                            emit_pv(*pend.pop(0))
                        pm = ppool.tile([128, QB], BF16,
                                        name=f"pm_{tag}_{qb}_{mc}", tag="pm",
                                        bufs=6)
                        nc.scalar.activation(pm, sc, AF.Exp)
                        pmm = ppool.tile([128, QB], BF16,
                                         name=f"pmm_{tag}_{qb}_{mc}", tag="pmm")
                        nc.vector.tensor_mul(
                            pmm, pm,
                            mask_sb[:, mc * N + qb * QB: mc * N + qb * QB + QB])
                        pend.append((pmm, mc))
                        if feed and qb * NT + mc >= feed_start:
                            feed.pop(0)()
                    for pv in pend:
                        emit_pv(*pv)
                    if qb == 0 and delay_cb0:
                        cb0_args = (qb, ot)
                    else:
                        out_cb(qb, ot)
                for th in feed:
                    th()
                if cb0_args is not None:
                    out_cb(*cb0_args)

            # ---- L1 prologue: head 0 aug only (heads 1-3 interleave into
            # the passes so their DVE work overlaps attention) ------------------
            catT = [const.tile([128, N], F32, name=f"catT{t}") for t in range(HD // 128)]

            def l1_proj(half, h):
                j0 = half * QB
                ps = pso.tile([65, QB], F32, name=f"prj_{h}_{half}", tag="ot")
                for nb in range(QB // 512):
                    nc.tensor.matmul(ps[:, nb * 512:(nb + 1) * 512],
                                     whT_sb[:, h * 66:h * 66 + 65],
                                     xT[:, j0 + nb * 512:j0 + (nb + 1) * 512],
                                     start=True, stop=True)
                return ps

            augs = [make_aug(f"l1h{h}", full65=True,
                             host_r2_row=negr2_d[h:h + 1, :],
                             dve_k=(h == 0)) for h in range(H)]

            def head_units(h):
                _, half_units = augs[h]

                def hmk(dst, mc, h=h):
                    nc.tensor.matmul(dst, xT[:, mc * 128:(mc + 1) * 128],
                                     whT_sb[:, h * 66:(h + 1) * 66],
                                     start=True, stop=True)

                us = []
                for half in range(NQB):
                    us += half_units(half,
                                     lambda half=half, h=h: l1_proj(half, h),
                                     hmk)
                return us

            for u in head_units(0):
                u()
            for mc in range(2, NT):
                mask_dma(mc)

            # ---- L1 passes ----------------------------------------------------
            def make_l1_cb(h):
                def l1_cb(qb, ot):
                    # one fast DVE copy releases the PSUM accumulator early —
                    # holding it through the DMA roundtrip blocks the next
                    # stage's PSUM tiles in the pool ring.
                    o1 = work.tile([65, QB], F32, name=f"o1_{h}_{qb}", tag="o1")
                    nc.vector.tensor_copy(o1, ot)
                    # denominators: row -> DRAM -> [128, QB/128] gather so the
                    # reciprocal runs on all lanes, then broadcast back.
                    rd = drb.tile([1, QB], F32, name=f"rd_{h}_{qb}", tag="rd")
                    nc.sync.dma_start(rd, o1[64:65, :])
                    dn = small.tile([128, QB // 128], F32, name=f"dn_{h}_{qb}",
                                    tag="dn")
                    nc.sync.dma_start(dn, rd.rearrange("o (c p) -> p (o c)", p=128))
                    rc = small.tile([128, QB // 128], F32, name=f"rc_{h}_{qb}",
                                    tag="rc")
                    nc.vector.reciprocal(rc, dn)
                    rd2 = drb.tile([1, QB], F32, name=f"rd2_{h}_{qb}", tag="rd2")
                    nc.sync.dma_start(rd2.rearrange("o (c p) -> p (o c)", p=128), rc)
                    recb = work.tile([64, QB], F32, name=f"rcb_{h}_{qb}", tag="rcb")
                    nc.sync.dma_start(recb, rd2.to_broadcast([64, QB]))
                    dst = catT[h // 2][(h % 2) * 64:(h % 2) * 64 + 64,
                                      qb * QB:(qb + 1) * QB]
                    nc.vector.tensor_mul(dst, o1[0:64, :], recb)
                    if use_bh:
                        nc.vector.tensor_scalar_add(
                            dst, dst, bh_cols[(h % 2) * 64:(h % 2) * 64 + 64,
                                              h // 2:h // 2 + 1])
                return l1_cb

            # ---- L2 build pieces (leaky + projection + aug), thunk-granular --
            zT = [const.tile([128, N], F32R, name=f"zT{t}") for t in range(HD // 128)]
            (aug_q2, aug_k2, H_aug2), half_units2 = make_aug(
                "l2", full65=False, r64_eng=nc.scalar)

            def l2_proj(half):
                j0 = half * QB
                ps = pso.tile([65, QB], F32, name=f"prj2_{half}", tag="ot")
                for nb in range(QB // 512):
                    for kc in range(2):
                        nc.tensor.matmul(
                            ps[0:64, nb * 512:(nb + 1) * 512],
                            woT_sb[:, kc * D:(kc + 1) * D],
                            zT[kc][:, j0 + nb * 512:j0 + (nb + 1) * 512],
                            start=(kc == 0), stop=(kc == 1))
                return ps

            def l2_hmk(dst, mc):
                for kc in range(2):
                    nc.tensor.matmul(dst, zT[kc][:, mc * 128:(mc + 1) * 128],
                                     woT_sb[:, kc * D:(kc + 1) * D],
                                     start=(kc == 0), stop=(kc == 1))

            def l2_units(half):
                us = []
                for part in (2 * half, 2 * half + 1):
                    def u_leak(part=part):
                        j0 = part * 512
                        for t in range(HD // 128):
                            nc.vector.scalar_tensor_tensor(
                                zT[t][:, j0:j0 + 512], catT[t][:, j0:j0 + 512],
                                0.2, catT[t][:, j0:j0 + 512],
                                op0=ALU.mult, op1=ALU.max)
                    us.append(u_leak)
                us += half_units2(half, lambda half=half: l2_proj(half), l2_hmk)
                return us

            # ---- L1 passes: head h+1's aug (or the first L2 half) feeds in
            # one thunk per chunk so its PE/DVE work hides under attention.
            for h in range(H):
                (aug_q, aug_k, H_aug), _ = augs[h]
                if h < H - 1:
                    feed, fs = head_units(h + 1), 0
                else:
                    feed, fs = l2_units(0), 3 * NT // 2
                attention(aug_q, aug_k, H_aug, make_l1_cb(h), f"l1h{h}",
                          feed=feed, feed_start=fs)

            # ---- L2 + epilogue ------------------------------------------------
            def l2_cb(qb, ot):
                NB = QB // 128
                # bounce the accumulator to SBUF (transpose input must be
                # SBUF; GPSIMD can't read PSUM so this rides DVE)
                o1 = work.tile([65, QB], F32, name=f"l2o1_{qb}", tag="o1")
                nc.vector.tensor_copy(o1, ot)
                # numerators AND denominator row -> [q, d|denom] via PE
                # transposes; per-block reciprocal straight off col 64 (no
                # DMA roundtrip needed in this layout).
                TW = 65
                tpb = work.tile([128, NB * TW], F32, name=f"tpb_{qb}", tag="tpb")
                ctr = work.tile([128, NB * D], F32, name=f"ctr_{qb}", tag="ctr")
                vars_ = small.tile([128, NB], F32, name=f"vars_{qb}", tag="vars")
                vscr = small.tile([128, 64], F32, name=f"vscr_{qb}", tag="vscr")
                for j in range(NB):
                    ftp = pss.tile([128, TW], F32, name=f"ftp_{qb}_{j}", tag="sc")
                    nc.tensor.transpose(ftp, o1[0:65, j * 128:(j + 1) * 128],
                                        idf32[:65, :65])
                    nc.vector.tensor_copy(tpb[:, j * TW:(j + 1) * TW], ftp)
                    rcj = small.tile([128, 1], F32, name=f"rcj_{qb}_{j}",
                                     tag="rcj")
                    nc.vector.reciprocal(rcj, tpb[:, j * TW + 64:(j + 1) * TW])
                    o2n = small.tile([128, D], F32, name=f"o2n_{qb}_{j}", tag="fo2")
                    nc.vector.tensor_scalar(o2n, tpb[:, j * TW:j * TW + D],
                                            rcj, None, op0=ALU.mult)
                    if use_bo:
                        nc.vector.tensor_add(o2n, o2n, bo_row)
                    z2 = small.tile([128, D], F32, name=f"z2_{qb}_{j}", tag="fz2")
                    nc.vector.scalar_tensor_tensor(z2, o2n, 0.2, o2n,
                                                   op0=ALU.mult, op1=ALU.max)
                    s1 = small.tile([128, 1], F32, name=f"s1_{qb}_{j}", tag="fs1")
                    nc.vector.tensor_reduce(s1, z2, axis=AX.X, op=ALU.add)
                    mu = small.tile([128, 1], F32, name=f"mu_{qb}_{j}", tag="fmu")
                    nc.vector.tensor_scalar_mul(mu, s1, 1.0 / D)
                    cj = ctr[:, j * D:(j + 1) * D]
                    nc.vector.tensor_scalar(cj, z2, mu, None, op0=ALU.subtract)
                    nc.vector.tensor_mul(vscr, cj, cj)
                    nc.vector.tensor_reduce(vars_[:, j:j + 1], vscr,
                                            axis=AX.X, op=ALU.add)
                # vars_ holds sum(ctr^2) = D*var; rstd*sqrt(D) folds the 1/D
                # back in: out = ctr * sqrt(D)/sqrt(sumsq + D*eps).
                # rsqrt via exp(-0.5*ln(x)) keeps ACT in the exp/ln table set
                # (a Sqrt would force two ACT_TABLE_LOADs mid exp-stream).
                lnv = small.tile([128, NB], F32, name=f"lnv_{qb}", tag="stds")
                nc.scalar.activation(lnv, vars_, AF.Ln, bias=eps_col)
                rstds = small.tile([128, NB], F32, name=f"rstds_{qb}", tag="rstds")
                nc.scalar.activation(rstds, lnv, AF.Exp, scale=-0.5)
                on = work.tile([128, NB * D], F32, name=f"on_{qb}", tag="on")
                for j in range(NB):
                    oj = on[:, j * D:(j + 1) * D]
                    nc.vector.tensor_scalar(oj, ctr[:, j * D:(j + 1) * D],
                                            rstds[:, j:j + 1], float(np.sqrt(D)),
                                            op0=ALU.mult, op1=ALU.mult)
                    if use_gamma:
                        nc.vector.tensor_mul(oj, oj, gamma_row)
                    if use_beta:
                        nc.vector.tensor_add(oj, oj, beta_row)
                # one DMA per qb: element (p, j, d) -> out[qb*QB + j*128 + p, d]
                odst = bass.AP(tensor=out_d, offset=qb * QB * D,
                               ap=[[D, 128], [128 * D, NB], [1, D]])
                nc.sync.dma_start(odst, on)

            attention(aug_q2, aug_k2, H_aug2, l2_cb, "l2",
                      feed=l2_units(1), feed_start=0)

    return nc


# ---------------------------------------------------------------------------
# Host-side runner (cached compiled executable via bass2jax/PJRT)
# ---------------------------------------------------------------------------
_RUNNER_CACHE = {}


def _make_runner(nc, n_cores):
    import jax
    from jax.sharding import Mesh, PartitionSpec
    from jax.experimental.shard_map import shard_map
    from concourse import bass2jax
    from concourse.bass2jax import _bass_exec_p, install_neuronx_cc_hook

    install_neuronx_cc_hook()
    partition_name = nc.partition_id_tensor.name if nc.partition_id_tensor else None

    in_names, out_names, out_avals = [], [], []
    for alloc in nc.m.functions[0].allocations:
        if not isinstance(alloc, mybir.MemoryLocationSet):
            continue
        name = alloc.memorylocations[0].name
        if alloc.kind == "ExternalInput":
            if name != partition_name:
                in_names.append(name)
        elif alloc.kind == "ExternalOutput":
            out_names.append(name)
            out_avals.append(jax.core.ShapedArray(tuple(alloc.tensor_shape),
                                                  mybir.dt.np(alloc.dtype)))
    n_params = len(in_names)
    n_outs = len(out_avals)
    all_in_names = list(in_names) + list(out_names)
    if partition_name is not None:
        all_in_names.append(partition_name)

    def _body(*args):
        operands = list(args)
        if partition_name is not None:
            operands.append(bass2jax.partition_id_tensor())
        outs = _bass_exec_p.bind(
            *operands,
            out_avals=tuple(out_avals),
            in_names=tuple(all_in_names),
            out_names=tuple(out_names),
            lowering_input_output_aliases=(),
            sim_require_finite=True,
            sim_require_nnan=True,
            nc=nc,
        )
        return tuple(outs)

    donate = tuple(range(n_params, n_params + n_outs))

    if n_cores == 1:
        jitted = jax.jit(_body, donate_argnums=donate, keep_unused=True)

        def run(in_maps):
            args = [np.asarray(in_maps[0][n]) for n in in_names]
            zeros = [np.zeros(a.shape, a.dtype) for a in out_avals]
            outs = jitted(*args, *zeros)
            jax.block_until_ready(outs)
            return [{n: np.asarray(outs[i]) for i, n in enumerate(out_names)}]

        return run

    devices = jax.devices()[:n_cores]
    mesh = Mesh(np.asarray(devices), ("core",))
    in_specs = (PartitionSpec("core"),) * (n_params + n_outs)
    out_specs = (PartitionSpec("core"),) * n_outs
    jitted = jax.jit(
        shard_map(_body, mesh=mesh, in_specs=in_specs, out_specs=out_specs,
                  check_rep=False),
        donate_argnums=donate,
        keep_unused=True,
    )

    def run(in_maps):
        per_core = [[np.asarray(m[n]) for n in in_names] for m in in_maps]
        concat_in = [np.concatenate([per_core[c][i] for c in range(n_cores)], axis=0)
                     for i in range(n_params)]
        concat_zero = [np.zeros((a.shape[0] * n_cores,) + a.shape[1:], a.dtype)
                       for a in out_avals]
        outs = jitted(*concat_in, *concat_zero)
        jax.block_until_ready(outs)
        results = []
        for c in range(n_cores):
            d = {}
            for i, n in enumerate(out_names):
                per_len = out_avals[i].shape[0]
                d[n] = np.asarray(outs[i][c * per_len:(c + 1) * per_len])
            results.append(d)
        return results

    return run


def _get_runner(flags, n_cores):
    key = (flags, n_cores)
    if key not in _RUNNER_CACHE:
        nc = build_gat(use_bh=flags[0], use_bo=flags[1],
                       use_gamma=flags[2], use_beta=flags[3])
        _RUNNER_CACHE[key] = (_make_runner(nc, n_cores), nc)
    return _RUNNER_CACHE[key][0]


def make_in_maps(x, graph, Wh, bh, Wo, bo, gamma, beta):
    B, N, C = x.shape
    H, D, _ = Wh.shape
    flags = (bool(np.any(bh)), bool(np.any(bo)),
             bool(np.any(gamma != 1.0)), bool(np.any(beta)))
    mask = (graph + np.eye(N, dtype=graph.dtype)) > 0
    maskt = np.ascontiguousarray(mask.T).astype(ml_dtypes.bfloat16)
    # augmented projection weights: per head 65 output cols; col 64 selects
    # the ones row of x^T so the proj PSUM carries the ones row for free.
    wht = np.zeros((C + 1, H * 66), np.float32)
    for h in range(H):
        wht[:C, h * 66:h * 66 + D] = Wh[h].T            # [c, d]
        wht[C, h * 66 + 64] = 1.0
    # woT_sb[p, kc*D+d] = Wo[d, kc*128+p]
    wot = np.ascontiguousarray(
        Wo.T.reshape(2, 128, D).transpose(1, 0, 2).reshape(128, 2 * D)).astype(np.float32)
    in_maps = []
    for b in range(B):
        xa = np.concatenate([x[b].T, np.ones((1, N), np.float32)], axis=0)
        hb = np.einsum('nc,hdc->hnd', x[b], Wh)            # [H, N, D]
        negr2 = -np.square(hb).sum(-1)                      # [H, N]
        m = {"xt": np.ascontiguousarray(xa).astype(np.float32),
             "maskt": maskt, "wht": wht, "wot": wot,
             "negr2": negr2.astype(np.float16)}
        if flags[0]:
            m["bh"] = np.ascontiguousarray(
                np.asarray(bh, np.float32).reshape(-1).reshape(2, 128).T)
        if flags[1]:
            m["bo"] = np.asarray(bo, np.float32)
        if flags[2]:
            m["gamma"] = np.asarray(gamma, np.float32)
        if flags[3]:
            m["beta"] = np.asarray(beta, np.float32)
        in_maps.append(m)
    return in_maps, flags


def kernel(x, graph, Wh, bh, Wo, bo, gamma, beta):
    x = np.asarray(x)
    B = x.shape[0]
    in_maps, flags = make_in_maps(np.asarray(x, np.float32), np.asarray(graph),
                                  np.asarray(Wh, np.float32),
                                  np.asarray(bh, np.float32),
                                  np.asarray(Wo, np.float32),
                                  np.asarray(bo, np.float32),
                                  np.asarray(gamma, np.float32),
                                  np.asarray(beta, np.float32))
    run = _get_runner(flags, B)
    results = run(in_maps)
    return np.stack([r["out"] for r in results], axis=0)


# revision 68
# speedup vs baseline: 1.1565x; 1.0050x over previous
"""GAT spatial kernel for trn2 (nn_GATSpatial_36112085025002) — v2.

Strategy
--------
Data-parallel over B=8 across the 8 NeuronCores; each core runs the full
2-layer GAT for one batch element.

v2 design (vs v1 baseline at ~455us):
  - ACT engine does ONLY the exp over the N^2 scores (the hard floor);
    all copies move to DVE/Pool/DMA.
  - Scores in transposed layout sT[keys, q] via K=65 augmented contraction:
    stationary aug_k rows 0-63 = h^T fp16, row 64 = ones (produced for free
    by an augmented projection: x^T gets a ones row, Wh^T gets a selector
    column); moving aug_q rows 0-63 = h^T, row 64 = -||h_q||^2 (softmax
    shift; exact by shift-invariance, so fp16 precision is fine).
  - -r^2 per chunk via one fused tensor_tensor_reduce off the PV-stationary
    tile, then PE-transpose + one strided DMA into aug_q row 64.
  - H_aug (PV stationary, [128, 65] per chunk incl ones col for free
    denominators) built with DMA-xbar transposes — zero PSUM traffic, so
    the whole L1 aug prep runs in a prologue overlapped with the mask DMA
    and the attention passes run back-to-back with no PE gaps (keeps the
    PE p-state ramped).
  - exp: ACT [128,1024] PSUM->bf16; mask as bf16 multiply on DVE (2x mode).
  - PSUM: scores [128,1024]x2 + out accum [65,1024]x2 = exactly 8 banks.
  - L1 normalization: denominators DMA-roundtrip (DRAM gather -> 128-lane
    reciprocal -> broadcast), numerators scaled straight out of PSUM on DVE.
  - L2 epilogue: numerators -> bf16 -> DMA-xbar transpose to [q, d], per-
    block reciprocal from a gathered [128, 8] column layout, leaky+LN on
    DVE with batched sqrt on ACT.
"""
import sys

sys.path.insert(0, '/opt/trn_rl_repo')

import numpy as np
import ml_dtypes

import concourse.bass as bass
import concourse.tile as tile
import concourse.mybir as mybir
from concourse.masks import make_identity

F32 = mybir.dt.float32
F32R = mybir.dt.float32r
F16 = mybir.dt.float16
BF16 = mybir.dt.bfloat16
AF = mybir.ActivationFunctionType
ALU = mybir.AluOpType
AX = mybir.AxisListType

N_CORES = 8
LN_EPS = 1e-5

# ---------------------------------------------------------------------------
# walrus workaround: this compiler build rejects >1 sync-wait per instruction.
# Split extra waits into standalone EventSemaphore instructions.
# ---------------------------------------------------------------------------
_orig_commit = tile.TileContext._commit_and_lower


def _patched_commit(self, inst, *args, **kwargs):
    si = getattr(inst, "sync_info", None)
    waits = list(si.on_wait) if si is not None and si.on_wait else []
    if len(waits) > 1:
        for w in waits[:-1]:
            ev = mybir.InstEventSemaphore(
                name=self.nc.get_next_instruction_name(),
                engine=inst.engine,
                ins=[],
                outs=[],
                sync_info=mybir.SyncInfo(on_wait=[w], on_update=[]),
            )
            _orig_commit(self, ev, *args, **kwargs)
        si.on_wait = [waits[-1]]
        inst.sync_info = si
    return _orig_commit(self, inst, *args, **kwargs)


def _patched_drain_and_barrier(self, tick_clock, wait_clock):
    from concourse.tile import ScopedClock

    nc = self.nc
    dummy = mybir.InstDrain(
        name="tail-drain-waits", ins=[], outs=[], bass_is_fusable=False
    )
    dummy.engine = nc.sync.engine
    wait_clock.add_sem_waits(dummy, ScopedClock({None: tick_clock.global_clock}))
    waits = list(dummy.sync_info.on_wait) if dummy.sync_info else []
    for w in waits:
        ev = mybir.InstEventSemaphore(
            name=nc.get_next_instruction_name(),
            engine=nc.sync.engine,
            ins=[],
            outs=[],
            sync_info=mybir.SyncInfo(on_wait=[w], on_update=[]),
        )
        nc.sync.add_instruction(ev)
    nc.sync.drain()

    nc.all_engine_barrier()
    assert self.sems is not None
    popped = nc._tile_sem_poison_stack.pop()
    assert popped is self._sem_poison
    nc.clear_and_free_semaphores(list(self.sems.allocated().values()))
    nc.all_engine_barrier()


if getattr(tile.TileContext, "_wait_split_patched", False) is False:
    tile.TileContext._commit_and_lower = _patched_commit
    tile.TileContext._drain_and_barrier = _patched_drain_and_barrier
    tile.TileContext._wait_split_patched = True


# ---------------------------------------------------------------------------
# Kernel builder
# ---------------------------------------------------------------------------
def build_gat(N=2048, C=64, H=4, D=64,
              use_bh=False, use_bo=False, use_gamma=False, use_beta=False):
    assert N % 128 == 0
    NT = N // 128                     # key chunks
    QB = 1024                         # q block
    NQB = N // QB
    HD = H * D
    CHW = 65                          # H_aug per-chunk column stride (64 + ones)

    nc = bass.Bass(trn_type="TRN2")
    xt_d = nc.dram_tensor("xt", [C + 1, N], F32R, kind="ExternalInput")
    maskt_d = nc.dram_tensor("maskt", [N, N], BF16, kind="ExternalInput")
    wht_d = nc.dram_tensor("wht", [C + 1, H * 66], F32R, kind="ExternalInput")
    negr2_d = nc.dram_tensor("negr2", [H, N], F16, kind="ExternalInput")
    wot_d = nc.dram_tensor("wot", [128, (HD // 128) * D], F32R, kind="ExternalInput")
    bh_d = nc.dram_tensor("bh", [128, HD // 128], F32, kind="ExternalInput") if use_bh else None
    bo_d = nc.dram_tensor("bo", [D], F32, kind="ExternalInput") if use_bo else None
    gamma_d = nc.dram_tensor("gamma", [D], F32, kind="ExternalInput") if use_gamma else None
    beta_d = nc.dram_tensor("beta", [D], F32, kind="ExternalInput") if use_beta else None
    out_d = nc.dram_tensor("out", [N, D], F32, kind="ExternalOutput")

    with tile.TileContext(nc) as tc:
        import contextlib
        ctx = contextlib.ExitStack()
        with ctx:
            const = ctx.enter_context(tc.tile_pool(name="const", bufs=1))
            aug = ctx.enter_context(tc.tile_pool(name="aug", bufs=3))
            work = ctx.enter_context(tc.tile_pool(name="work", bufs=2))
            small = ctx.enter_context(tc.tile_pool(name="small", bufs=4))
            ppool = ctx.enter_context(tc.tile_pool(name="ppool", bufs=4))
            pss = ctx.enter_context(tc.tile_pool(name="pss", bufs=2, space="PSUM"))
            drb = ctx.enter_context(tc.tile_pool(name="drb", bufs=4, space="DRAM"))
            pso = ctx.enter_context(tc.tile_pool(name="pso", bufs=2, space="PSUM"))

            # ---- constants ----------------------------------------------------
            idf32 = const.tile([128, 128], F32, name="idf32")
            make_identity(nc, idf32)
            idf16 = const.tile([128, 128], F16, name="idf16")
            nc.vector.tensor_copy(idf16, idf32)
            eps_col = const.tile([128, 1], F32, name="eps_col")
            nc.vector.memset(eps_col, LN_EPS * D)
            onesrow = const.tile([1, N], F16, name="onesrow")
            nc.gpsimd.memset(onesrow, 1.0)

            xT = const.tile([C + 1, N], F32R, name="xT")
            nc.sync.dma_start(xT, xt_d[:, :])
            whT_sb = const.tile([C + 1, H * 66], F32R, name="whT_sb")
            nc.sync.dma_start(whT_sb, wht_d[:, :])
            woT_sb = const.tile([128, 2 * D], F32R, name="woT_sb")
            nc.sync.dma_start(woT_sb, wot_d[:, :])

            # mask resident in SBUF: [128, NT*N] bf16, chunk mc at cols
            # [mc*N, (mc+1)*N); one DMA per chunk, split across both hwdge
            # queues (SP + ACT) in consumption order. Chunks 2+ are emitted
            # after the head-0 aug build so its r64 DMA isn't queued behind
            # them on SP (see below).
            mask_sb = const.tile([128, NT * N], BF16, name="mask_sb")

            def mask_dma(mc):
                eng = nc.sync if mc % 2 == 0 else nc.scalar
                eng.dma_start(mask_sb[:, mc * N:(mc + 1) * N],
                              maskt_d[mc * 128:(mc + 1) * 128, :])
            mask_dma(0)
            mask_dma(1)

            bh_cols = None
            if use_bh:
                bh_cols = const.tile([128, 2], F32, name="bh_cols")
                nc.sync.dma_start(bh_cols, bh_d[:, :])
            bo_row = gamma_row = beta_row = None
            if use_bo:
                bo_row = const.tile([128, D], F32, name="bo_row")
                nc.sync.dma_start(bo_row, bo_d.to_broadcast([128, D]))
            if use_gamma:
                gamma_row = const.tile([128, D], F32, name="gamma_row")
                nc.sync.dma_start(gamma_row, gamma_d.to_broadcast([128, D]))
            if use_beta:
                beta_row = const.tile([128, D], F32, name="beta_row")
                nc.sync.dma_start(beta_row, beta_d.to_broadcast([128, D]))

            # ---- aug builder --------------------------------------------------
            HNT = NT // 2             # chunks per half

            def make_aug(tag, full65, r64_eng=None, host_r2_row=None,
                         dve_k=False):
                """Returns ((aug_q, aug_k, H_aug), half_units). half_units(
                half, proj_thunk, hmk_thunk) -> list of small thunks building
                columns [half*QB,(half+1)*QB): d-major proj -> aug copies;
                key-major H chunks straight from mini-matmuls (no transposes);
                fused -r^2 per chunk; strided DMA into aug_q row 64. Thunk-
                granular so the work interleaves into a running pass."""
                aug_q = aug.tile([65, N], F16, name=f"aq_{tag}", tag="aug_q")
                aug_k = aug.tile([65, N], F16, name=f"ak_{tag}", tag="aug_k")
                H_aug = aug.tile([128, CHW * NT], F16, name=f"Ha_{tag}", tag="H_aug")
                rows = 65 if full65 else 64
                hw = 66 if full65 else 64   # k-major matmul output width
                # (66: f32r moving operands need an even free size; col 64 is
                # the ones column, col 65 zero padding)

                def half_units(half, proj_thunk, hmk_thunk):
                    j0 = half * QB
                    st = {}

                    def u_proj():
                        if half == 0:
                            if host_r2_row is not None:
                                # host-precomputed -r^2 straight into row 64
                                nc.sync.dma_start(aug_q[64:65, :], host_r2_row)
                            nc.sync.dma_start(aug_k[64:65, :], onesrow)
                        ps = proj_thunk()
                        # GPSIMD can't read PSUM: aug_q from PSUM on DVE,
                        # aug_k mirrored from aug_q on Pool (SBUF->SBUF).
                        nc.vector.tensor_copy(aug_q[0:64, j0:j0 + QB],
                                              ps[0:64, :])
                        keng = nc.vector if dve_k else nc.gpsimd
                        keng.tensor_copy(aug_k[0:64, j0:j0 + QB],
                                         aug_q[0:64, j0:j0 + QB])

                    HH = HNT // 2      # chunks per hp tile (PSUM bank limit)

                    def u_hmk(sub):
                        # separate PSUM tile per 4 chunks: a matmul output
                        # must not cross a 512-f32 PSUM bank boundary
                        st[f'hp{sub}'] = pso.tile([128, HH * hw], F32,
                                                  name=f"hp_{tag}_{half}_{sub}",
                                                  tag="ot")
                        for k in range(HH):
                            hmk_thunk(st[f'hp{sub}'][:, k * hw:(k + 1) * hw],
                                      half * HNT + sub * HH + k)

                    def u_hcopy():
                        for sub in range(2):
                            hp = st[f'hp{sub}']
                            h0 = (half * HNT + sub * HH) * CHW
                            if full65:
                                src = bass.AP(tensor=hp.tensor, offset=hp.offset,
                                              ap=[hp.ap[0], [hw, HH], [1, CHW]])
                                nc.vector.tensor_copy(
                                    H_aug[:, h0:h0 + HH * CHW], src)
                            else:
                                dst = bass.AP(tensor=H_aug.tensor,
                                              offset=H_aug.offset + h0,
                                              ap=[H_aug.ap[0], [CHW, HH], [1, 64]])
                                nc.vector.tensor_copy(dst, hp)
                                ones_col = bass.AP(
                                    tensor=H_aug.tensor,
                                    offset=H_aug.offset + h0 + 64,
                                    ap=[H_aug.ap[0], [CHW, HH]])
                                nc.vector.memset(ones_col, 1.0)
                        if host_r2_row is None:
                            st['negr2'] = small.tile([128, HNT], F32,
                                                     name=f"nr_{tag}_{half}",
                                                     tag="negr2")
                            st['scr'] = small.tile([128, 64], F16,
                                                   name=f"scr_{tag}_{half}",
                                                   tag="scr")

                    def u_r2(k2):
                        for k in (2 * k2, 2 * k2 + 1):
                            mc = half * HNT + k
                            nc.vector.tensor_mul(
                                st['scr'], H_aug[:, mc * CHW:mc * CHW + 64],
                                H_aug[:, mc * CHW:mc * CHW + 64])
                            nc.vector.tensor_reduce(
                                st['negr2'][:, k:k + 1], st['scr'],
                                axis=AX.X, op=ALU.add)

                    def u_tail():
                        negr2h = small.tile([128, HNT], F16,
                                            name=f"nrh_{tag}_{half}",
                                            tag="negr2h")
                        nc.vector.tensor_scalar_mul(negr2h, st['negr2'], -1.0)
                        ntp = pso.tile([HNT, 128], F16,
                                       name=f"ntp_{tag}_{half}", tag="ot")
                        nc.tensor.transpose(ntp, negr2h, idf16[:128, :128])
                        nrsb = small.tile([HNT, 128], F16,
                                          name=f"nrsb_{tag}_{half}", tag="nrsb")
                        nc.vector.tensor_copy(nrsb, ntp)
                        r64 = aug_q[64:65, :]
                        r64v = bass.AP(tensor=r64.tensor,
                                       offset=r64.offset + j0,
                                       ap=[r64.ap[0], [128, HNT], [1, 128]])
                        (r64_eng or nc.sync).dma_start(r64v, nrsb)

                    us = [u_proj, lambda: u_hmk(0), lambda: u_hmk(1), u_hcopy]
                    if host_r2_row is None:
                        us += [lambda k2=k2: u_r2(k2) for k2 in range(HNT // 2)]
                        us.append(u_tail)
                    return us

                return (aug_q, aug_k, H_aug), half_units

            # ---- attention core ----------------------------------------------
            # PV(mc) is emitted AFTER scores(mc+1): the PE queue is in-order,
            # so this keeps the next chunk's scores flowing while exp/mask of
            # the current chunk complete (PV parks in the wait queue).
            def attention(aug_q, aug_k, H_aug, out_cb, tag,
                          feed=None, feed_start=0, delay_cb0=False):
                """feed: list of thunks emitted one-per-chunk starting at
                global chunk index feed_start — lets the next stage's build
                work interleave between this pass's mask-mults without
                overflowing the engines' 4-deep wait queues."""
                feed = list(feed) if feed else []
                cb0_args = None
                for qb in range(NQB):
                    ot = pso.tile([65, QB], F32, name=f"ot_{tag}_{qb}", tag="ot")

                    def emit_pv(pmm, mc, ot=ot):
                        for nb in range(QB // 512):
                            nc.tensor.matmul(
                                ot[:, nb * 512:(nb + 1) * 512],
                                H_aug[:, mc * CHW:mc * CHW + 65],
                                pmm[:, nb * 512:(nb + 1) * 512],
                                start=(mc == 0), stop=(mc == NT - 1))

                    # PV lag 2: the critical cycle exp(k)->mask(k)->PV(k)->
                    # [PE in-order]->scores->exp then spans 3 chunks instead
                    # of 2, dropping the steady-state cadence to ~max(engine).
                    pend = []
                    for mc in range(NT):
                        sc = pss.tile([128, QB], F32,
                                      name=f"sc_{tag}_{qb}_{mc}", tag="sc")
                        for nb in range(QB // 512):
                            q0 = qb * QB + nb * 512
                            nc.tensor.matmul(
                                sc[:, nb * 512:(nb + 1) * 512],
                                aug_k[:, mc * 128:(mc + 1) * 128],
                                aug_q[:, q0:q0 + 512],
                                start=True, stop=True)
                        if len(pend) >= 2:
                            emit_pv(*pend.pop(0))
                        pm = ppool.tile([128, QB], BF16,
                                        name=f"pm_{tag}_{qb}_{mc}", tag="pm",
                                        bufs=6)
                        nc.scalar.activation(pm, sc, AF.Exp)
                        pmm = ppool.tile([128, QB], BF16,
                                         name=f"pmm_{tag}_{qb}_{mc}", tag="pmm")
                        nc.vector.tensor_mul(
                            pmm, pm,
                            mask_sb[:, mc * N + qb * QB: mc * N + qb * QB + QB])
                        pend.append((pmm, mc))
                        if feed and qb * NT + mc >= feed_start:
                            feed.pop(0)()
                    for pv in pend:
                        emit_pv(*pv)
                    if qb == 0 and delay_cb0:
                        cb0_args = (qb, ot)
                    else:
                        out_cb(qb, ot)
                for th in feed:
                    th()
                if cb0_args is not None:
                    out_cb(*cb0_args)

            # ---- L1 prologue: head 0 aug only (heads 1-3 interleave into
            # the passes so their DVE work overlaps attention) ------------------
            catT = [const.tile([128, N], F32, name=f"catT{t}") for t in range(HD // 128)]

            def l1_proj(half, h):
                j0 = half * QB
                ps = pso.tile([65, QB], F32, name=f"prj_{h}_{half}", tag="ot")
                for nb in range(QB // 512):
                    nc.tensor.matmul(ps[:, nb * 512:(nb + 1) * 512],
                                     whT_sb[:, h * 66:h * 66 + 65],
                                     xT[:, j0 + nb * 512:j0 + (nb + 1) * 512],
                                     start=True, stop=True)
                return ps

            augs = [make_aug(f"l1h{h}", full65=True,
                             host_r2_row=negr2_d[h:h + 1, :],
                             dve_k=(h == 0)) for h in range(H)]

            def head_units(h):
                _, half_units = augs[h]

                def hmk(dst, mc, h=h):
                    nc.tensor.matmul(dst, xT[:, mc * 128:(mc + 1) * 128],
                                     whT_sb[:, h * 66:(h + 1) * 66],
                                     start=True, stop=True)

                us = []
                for half in range(NQB):
                    us += half_units(half,
                                     lambda half=half, h=h: l1_proj(half, h),
                                     hmk)
                return us

            for u in head_units(0):
                u()
            for mc in range(2, NT):
                mask_dma(mc)

            # ---- L1 passes ----------------------------------------------------
            def make_l1_cb(h):
                def l1_cb(qb, ot):
                    # one fast DVE copy releases the PSUM accumulator early —
                    # holding it through the DMA roundtrip blocks the next
                    # stage's PSUM tiles in the pool ring.
                    o1 = work.tile([65, QB], F32, name=f"o1_{h}_{qb}", tag="o1")
                    nc.vector.tensor_copy(o1, ot)
                    # denominators: row -> DRAM -> [128, QB/128] gather so the
                    # reciprocal runs on all lanes, then broadcast back.
                    rd = drb.tile([1, QB], F32, name=f"rd_{h}_{qb}", tag="rd")
                    nc.sync.dma_start(rd, o1[64:65, :])
                    dn = small.tile([128, QB // 128], F32, name=f"dn_{h}_{qb}",
                                    tag="dn")
                    nc.sync.dma_start(dn, rd.rearrange("o (c p) -> p (o c)", p=128))
                    rc = small.tile([128, QB // 128], F32, name=f"rc_{h}_{qb}",
                                    tag="rc")
                    nc.vector.reciprocal(rc, dn)
                    rd2 = drb.tile([1, QB], F32, name=f"rd2_{h}_{qb}", tag="rd2")
                    nc.sync.dma_start(rd2.rearrange("o (c p) -> p (o c)", p=128), rc)
                    recb = work.tile([64, QB], F32, name=f"rcb_{h}_{qb}", tag="rcb")
                    nc.sync.dma_start(recb, rd2.to_broadcast([64, QB]))
                    dst = catT[h // 2][(h % 2) * 64:(h % 2) * 64 + 64,
                                      qb * QB:(qb + 1) * QB]
                    nc.vector.tensor_mul(dst, o1[0:64, :], recb)
                    if use_bh:
                        nc.vector.tensor_scalar_add(
                            dst, dst, bh_cols[(h % 2) * 64:(h % 2) * 64 + 64,
                                              h // 2:h // 2 + 1])
                return l1_cb

            # ---- L2 build pieces (leaky + projection + aug), thunk-granular --
            zT = [const.tile([128, N], F32R, name=f"zT{t}") for t in range(HD // 128)]
            (aug_q2, aug_k2, H_aug2), half_units2 = make_aug(
                "l2", full65=False, r64_eng=nc.scalar)

            def l2_proj(half):
                j0 = half * QB
                ps = pso.tile([65, QB], F32, name=f"prj2_{half}", tag="ot")
                for nb in range(QB // 512):
                    for kc in range(2):
                        nc.tensor.matmul(
                            ps[0:64, nb * 512:(nb + 1) * 512],
                            woT_sb[:, kc * D:(kc + 1) * D],
                            zT[kc][:, j0 + nb * 512:j0 + (nb + 1) * 512],
                            start=(kc == 0), stop=(kc == 1))
                return ps

            def l2_hmk(dst, mc):
                for kc in range(2):
                    nc.tensor.matmul(dst, zT[kc][:, mc * 128:(mc + 1) * 128],
                                     woT_sb[:, kc * D:(kc + 1) * D],
                                     start=(kc == 0), stop=(kc == 1))

            def l2_units(half):
                us = []
                for part in (2 * half, 2 * half + 1):
                    def u_leak(part=part):
                        j0 = part * 512
                        for t in range(HD // 128):
                            nc.vector.scalar_tensor_tensor(
                                zT[t][:, j0:j0 + 512], catT[t][:, j0:j0 + 512],
                                0.2, catT[t][:, j0:j0 + 512],
                                op0=ALU.mult, op1=ALU.max)
                    us.append(u_leak)
                us += half_units2(half, lambda half=half: l2_proj(half), l2_hmk)
                return us

            # ---- L1 passes: head h+1's aug (or the first L2 half) feeds in
            # one thunk per chunk so its PE/DVE work hides under attention.
            for h in range(H):
                (aug_q, aug_k, H_aug), _ = augs[h]
                if h < H - 1:
                    feed, fs = head_units(h + 1), 0
                else:
                    feed, fs = l2_units(0), 3 * NT // 2
                attention(aug_q, aug_k, H_aug, make_l1_cb(h), f"l1h{h}",
                          feed=feed, feed_start=fs)

            # ---- L2 + epilogue ------------------------------------------------
            def l2_cb(qb, ot):
                NB = QB // 128
                # bounce the accumulator to SBUF (transpose input must be
                # SBUF; GPSIMD can't read PSUM so this rides DVE)
                o1 = work.tile([65, QB], F32, name=f"l2o1_{qb}", tag="o1")
                nc.vector.tensor_copy(o1, ot)
                # numerators AND denominator row -> [q, d|denom] via PE
                # transposes; per-block reciprocal straight off col 64 (no
                # DMA roundtrip needed in this layout).
                TW = 65
                tpb = work.tile([128, NB * TW], F32, name=f"tpb_{qb}", tag="tpb")
                ctr = work.tile([128, NB * D], F32, name=f"ctr_{qb}", tag="ctr")
                vars_ = small.tile([128, NB], F32, name=f"vars_{qb}", tag="vars")
                vscr = small.tile([128, 64], F32, name=f"vscr_{qb}", tag="vscr")
                for j in range(NB):
                    ftp = pss.tile([128, TW], F32, name=f"ftp_{qb}_{j}", tag="sc")
                    nc.tensor.transpose(ftp, o1[0:65, j * 128:(j + 1) * 128],
                                        idf32[:65, :65])
                    nc.vector.tensor_copy(tpb[:, j * TW:(j + 1) * TW], ftp)
                    rcj = small.tile([128, 1], F32, name=f"rcj_{qb}_{j}",
                                     tag="rcj")
                    nc.vector.reciprocal(rcj, tpb[:, j * TW + 64:(j + 1) * TW])
                    o2n = small.tile([128, D], F32, name=f"o2n_{qb}_{j}", tag="fo2")
                    nc.vector.tensor_scalar(o2n, tpb[:, j * TW:j * TW + D],
                                            rcj, None, op0=ALU.mult)
                    if use_bo:
                        nc.vector.tensor_add(o2n, o2n, bo_row)
                    z2 = small.tile([128, D], F32, name=f"z2_{qb}_{j}", tag="fz2")
                    nc.vector.scalar_tensor_tensor(z2, o2n, 0.2, o2n,
                                                   op0=ALU.mult, op1=ALU.max)
                    s1 = small.tile([128, 1], F32, name=f"s1_{qb}_{j}", tag="fs1")
                    nc.vector.tensor_reduce(s1, z2, axis=AX.X, op=ALU.add)
                    mu = small.tile([128, 1], F32, name=f"mu_{qb}_{j}", tag="fmu")
                    nc.vector.tensor_scalar_mul(mu, s1, 1.0 / D)
                    cj = ctr[:, j * D:(j + 1) * D]
                    nc.vector.tensor_scalar(cj, z2, mu, None, op0=ALU.subtract)
                    nc.vector.tensor_mul(vscr, cj, cj)
                    nc.vector.tensor_reduce(vars_[:, j:j + 1], vscr,
                                            axis=AX.X, op=ALU.add)
                # vars_ holds sum(ctr^2) = D*var; rstd*sqrt(D) folds the 1/D
                # back in: out = ctr * sqrt(D)/sqrt(sumsq + D*eps).
                # rsqrt via exp(-0.5*ln(x)) keeps ACT in the exp/ln table set
                # (a Sqrt would force two ACT_TABLE_LOADs mid exp-stream).
                lnv = small.tile([128, NB], F32, name=f"lnv_{qb}", tag="stds")
                nc.scalar.activation(lnv, vars_, AF.Ln, bias=eps_col)
                rstds = small.tile([128, NB], F32, name=f"rstds_{qb}", tag="rstds")
                nc.scalar.activation(rstds, lnv, AF.Exp, scale=-0.5)
                on = work.tile([128, NB * D], F32, name=f"on_{qb}", tag="on")
                for j in range(NB):
                    oj = on[:, j * D:(j + 1) * D]
                    nc.vector.tensor_scalar(oj, ctr[:, j * D:(j + 1) * D],
                                            rstds[:, j:j + 1], float(np.sqrt(D)),
                                            op0=ALU.mult, op1=ALU.mult)
                    if use_gamma:
                        nc.vector.tensor_mul(oj, oj, gamma_row)
                    if use_beta:
                        nc.vector.tensor_add(oj, oj, beta_row)
                # one DMA per qb: element (p, j, d) -> out[qb*QB + j*128 + p, d]
                odst = bass.AP(tensor=out_d, offset=qb * QB * D,
                               ap=[[D, 128], [128 * D, NB], [1, D]])
                nc.sync.dma_start(odst, on)

            attention(aug_q2, aug_k2, H_aug2, l2_cb, "l2",
                      feed=l2_units(1), feed_start=0)

    return nc


# ---------------------------------------------------------------------------
# Host-side runner (cached compiled executable via bass2jax/PJRT)
# ---------------------------------------------------------------------------
_RUNNER_CACHE = {}


def _make_runner(nc, n_cores):
    import jax
    from jax.sharding import Mesh, PartitionSpec
    from jax.experimental.shard_map import shard_map
    from concourse import bass2jax
    from concourse.bass2jax import _bass_exec_p, install_neuronx_cc_hook

    install_neuronx_cc_hook()
    partition_name = nc.partition_id_tensor.name if nc.partition_id_tensor else None

    in_names, out_names, out_avals = [], [], []
    for alloc in nc.m.functions[0].allocations:
        if not isinstance(alloc, mybir.MemoryLocationSet):
            continue
        name = alloc.memorylocations[0].name
        if alloc.kind == "ExternalInput":
            if name != partition_name:
                in_names.append(name)
        elif alloc.kind == "ExternalOutput":
            out_names.append(name)
            out_avals.append(jax.core.ShapedArray(tuple(alloc.tensor_shape),
                                                  mybir.dt.np(alloc.dtype)))
    n_params = len(in_names)
    n_outs = len(out_avals)
    all_in_names = list(in_names) + list(out_names)
    if partition_name is not None:
        all_in_names.append(partition_name)

    def _body(*args):
        operands = list(args)
        if partition_name is not None:
            operands.append(bass2jax.partition_id_tensor())
        outs = _bass_exec_p.bind(
            *operands,
            out_avals=tuple(out_avals),
            in_names=tuple(all_in_names),
            out_names=tuple(out_names),
            lowering_input_output_aliases=(),
            sim_require_finite=True,
            sim_require_nnan=True,
            nc=nc,
        )
        return tuple(outs)

    donate = tuple(range(n_params, n_params + n_outs))

    if n_cores == 1:
        jitted = jax.jit(_body, donate_argnums=donate, keep_unused=True)

        def run(in_maps):
            args = [np.asarray(in_maps[0][n]) for n in in_names]
            zeros = [np.zeros(a.shape, a.dtype) for a in out_avals]
            outs = jitted(*args, *zeros)
            jax.block_until_ready(outs)
            return [{n: np.asarray(outs[i]) for i, n in enumerate(out_names)}]

        return run

    devices = jax.devices()[:n_cores]
    mesh = Mesh(np.asarray(devices), ("core",))
    in_specs = (PartitionSpec("core"),) * (n_params + n_outs)
    out_specs = (PartitionSpec("core"),) * n_outs
    jitted = jax.jit(
        shard_map(_body, mesh=mesh, in_specs=in_specs, out_specs=out_specs,
                  check_rep=False),
        donate_argnums=donate,
        keep_unused=True,
    )

    def run(in_maps):
        per_core = [[np.asarray(m[n]) for n in in_names] for m in in_maps]
        concat_in = [np.concatenate([per_core[c][i] for c in range(n_cores)], axis=0)
                     for i in range(n_params)]
        concat_zero = [np.zeros((a.shape[0] * n_cores,) + a.shape[1:], a.dtype)
                       for a in out_avals]
        outs = jitted(*concat_in, *concat_zero)
        jax.block_until_ready(outs)
        results = []
        for c in range(n_cores):
            d = {}
            for i, n in enumerate(out_names):
                per_len = out_avals[i].shape[0]
                d[n] = np.asarray(outs[i][c * per_len:(c + 1) * per_len])
            results.append(d)
        return results

    return run


def _get_runner(flags, n_cores):
    key = (flags, n_cores)
    if key not in _RUNNER_CACHE:
        nc = build_gat(use_bh=flags[0], use_bo=flags[1],
                       use_gamma=flags[2], use_beta=flags[3])
        _RUNNER_CACHE[key] = (_make_runner(nc, n_cores), nc)
    return _RUNNER_CACHE[key][0]


def make_in_maps(x, graph, Wh, bh, Wo, bo, gamma, beta):
    B, N, C = x.shape
    H, D, _ = Wh.shape
    flags = (bool(np.any(bh)), bool(np.any(bo)),
             bool(np.any(gamma != 1.0)), bool(np.any(beta)))
    mask = (graph + np.eye(N, dtype=graph.dtype)) > 0
    maskt = np.ascontiguousarray(mask.T).astype(ml_dtypes.bfloat16)
    # augmented projection weights: per head 65 output cols; col 64 selects
    # the ones row of x^T so the proj PSUM carries the ones row for free.
    wht = np.zeros((C + 1, H * 66), np.float32)
    for h in range(H):
        wht[:C, h * 66:h * 66 + D] = Wh[h].T            # [c, d]
        wht[C, h * 66 + 64] = 1.0
    # woT_sb[p, kc*D+d] = Wo[d, kc*128+p]
    wot = np.ascontiguousarray(
        Wo.T.reshape(2, 128, D).transpose(1, 0, 2).reshape(128, 2 * D)).astype(np.float32)
    in_maps = []
    for b in range(B):
        xa = np.concatenate([x[b].T, np.ones((1, N), np.float32)], axis=0)
        hb = np.einsum('nc,hdc->hnd', x[b], Wh)            # [H, N, D]
        negr2 = -np.square(hb).sum(-1)                      # [H, N]
        m = {"xt": np.ascontiguousarray(xa).astype(np.float32),
             "maskt": maskt, "wht": wht, "wot": wot,
             "negr2": negr2.astype(np.float16)}
        if flags[0]:
            m["bh"] = np.ascontiguousarray(
                np.asarray(bh, np.float32).reshape(-1).reshape(2, 128).T)
        if flags[1]:
            m["bo"] = np.asarray(bo, np.float32)
        if flags[2]:
            m["gamma"] = np.asarray(gamma, np.float32)
        if flags[3]:
            m["beta"] = np.asarray(beta, np.float32)
        in_maps.append(m)
    return in_maps, flags


def kernel(x, graph, Wh, bh, Wo, bo, gamma, beta):
    x = np.asarray(x)
    B = x.shape[0]
    in_maps, flags = make_in_maps(np.asarray(x, np.float32), np.asarray(graph),
                                  np.asarray(Wh, np.float32),
                                  np.asarray(bh, np.float32),
                                  np.asarray(Wo, np.float32),
                                  np.asarray(bo, np.float32),
                                  np.asarray(gamma, np.float32),
                                  np.asarray(beta, np.float32))
    run = _get_runner(flags, B)
    results = run(in_maps)
    return np.stack([r["out"] for r in results], axis=0)
